# revision 17
# baseline (speedup 1.0000x reference)
"""BiMamba encoder block on 8 trn2 NeuronCores.

Sharding: core = (batch b in {0,1}) x (direction in {fwd,bwd}) x
(d_inner half in {0,1}).  Each core runs the same Bass program on its own
shard: LN1 -> in-proj -> depthwise causal conv -> silu -> x-proj ->
dt/softplus -> selective scan (native DVE tensor_tensor_scan per
(d-tile, n)) -> gated output projection partial.  Host sums the four
partials per batch (bwd cores process a host-flipped sequence) and
applies LN2 + w2 + exact GELU.
"""
import numpy as np

D_MODEL = 256
D_STATE = 64
D_CONV = 4
D_INNER = 512
DT_RANK = 16
BATCH = 2
SEQ = 1024
LN_EPS = 1e-5

HALF = D_INNER // 2  # 256 channels per core
P = 128
L = SEQ

_cache = {}



def _build(iters=1, variant="full"):
    import concourse.bacc as bacc
    import concourse.mybir as mybir
    from concourse.tile import TileContext

    f32 = mybir.dt.float32
    AF = mybir.ActivationFunctionType
    OP = mybir.AluOpType

    nc = bacc.Bacc("TRN2", target_bir_lowering=False, debug=False,
                   num_devices=8)

    # ---- per-core inputs (host-prepped) ----
    x_in = nc.declare_dram_parameter("x_in", [L, D_MODEL], f32, isOutput=False)
    in_wT = nc.declare_dram_parameter("in_wT", [D_MODEL, 768], f32,
                                      isOutput=False)  # cols: xi(512), z_half(256)
    xprojT = nc.declare_dram_parameter("xprojT", [D_INNER, 256], f32,
                                       isOutput=False)  # [dt16 B64 0*48 | C64 0*64]
    dt_wT = nc.declare_dram_parameter("dt_wT", [DT_RANK, HALF], f32,
                                      isOutput=False)
    conv_w = nc.declare_dram_parameter("conv_w", [D_INNER, D_CONV], f32,
                                       isOutput=False)
    conv_b = nc.declare_dram_parameter("conv_b", [D_INNER, 1], f32,
                                       isOutput=False)
    dt_b = nc.declare_dram_parameter("dt_b", [HALF, 1], f32, isOutput=False)
    A_in = nc.declare_dram_parameter("A_in", [HALF, D_STATE], f32,
                                     isOutput=False)
    Dp_in = nc.declare_dram_parameter("Dp_in", [HALF, 1], f32, isOutput=False)
    out_wT = nc.declare_dram_parameter("out_wT", [HALF, D_MODEL], f32,
                                       isOutput=False)
    ln1_g = nc.declare_dram_parameter("ln1_g", [D_MODEL, 1], f32,
                                      isOutput=False)
    ln1_b = nc.declare_dram_parameter("ln1_b", [D_MODEL, 1], f32,
                                      isOutput=False)
    ident = nc.declare_dram_parameter("ident", [P, P], f32, isOutput=False)
    ones1 = nc.declare_dram_parameter("ones1", [1, P], f32, isOutput=False)
    id64_in = nc.declare_dram_parameter("id64_in", [D_STATE, D_STATE], f32,
                                        isOutput=False)

    part = nc.declare_dram_parameter("part", [D_MODEL, L], f32, isOutput=True)

    from contextlib import nullcontext
    with TileContext(nc) as tc:
        with tc.tile_pool(name="wpool", bufs=1) as wp, \
             tc.tile_pool(name="xpool", bufs=1) as xp, \
             tc.tile_pool(name="work", bufs=2) as wk, \
             tc.tile_pool(name="psA", bufs=2, space="PSUM") as psA, \
             (tc.For_i(0, iters, 1) if iters > 1 else nullcontext()):

            # ---------- load weights ----------
            eps_c = wp.tile([P, 1], f32, name="eps_c")
            nc.gpsimd.memset(eps_c[:], LN_EPS)
            idt = wp.tile([P, P], f32, name="idt")
            nc.sync.dma_start(out=idt[:], in_=ident[:])
            ones_sb = wp.tile([1, P], f32, name="ones_sb")
            nc.sync.dma_start(out=ones_sb[:], in_=ones1[:])
            id64_sb = wp.tile([D_STATE, D_STATE], f32, name="id64_sb")
            nc.sync.dma_start(out=id64_sb[:], in_=id64_in[:])
            inw_sb = wp.tile([P, 2, 768], f32, name="inw_sb")  # [k-chunk][...]
            nc.sync.dma_start(
                out=inw_sb[:], in_=in_wT.rearrange("(a k) n -> k a n", a=2))
            xpj_sb = wp.tile([P, 4, 256], f32, name="xpj_sb")
            nc.sync.dma_start(
                out=xpj_sb[:], in_=xprojT.rearrange("(a k) n -> k a n", a=4))
            dtw_sb = wp.tile([DT_RANK, HALF], f32, name="dtw_sb")
            nc.sync.dma_start(out=dtw_sb[:], in_=dt_wT[:])
            cw_sb = wp.tile([P, 4, D_CONV], f32, name="cw_sb")
            nc.sync.dma_start(
                out=cw_sb[:], in_=conv_w.rearrange("(a k) n -> k a n", a=4))
            cb_sb = wp.tile([P, 4, 1], f32, name="cb_sb")
            nc.sync.dma_start(
                out=cb_sb[:], in_=conv_b.rearrange("(a k) n -> k a n", a=4))
            dtb_sb = wp.tile([P, 2, 1], f32, name="dtb_sb")
            nc.sync.dma_start(
                out=dtb_sb[:], in_=dt_b.rearrange("(a k) n -> k a n", a=2))
            A_sb = wp.tile([P, 2, D_STATE], f32, name="A_sb")
            nc.sync.dma_start(
                out=A_sb[:], in_=A_in.rearrange("(a k) n -> k a n", a=2))
            Dp_sb = wp.tile([P, 2, 1], f32, name="Dp_sb")
            nc.sync.dma_start(
                out=Dp_sb[:], in_=Dp_in.rearrange("(a k) n -> k a n", a=2))
            ow_sb = wp.tile([P, 2, D_MODEL], f32, name="ow_sb")
            nc.sync.dma_start(
                out=ow_sb[:], in_=out_wT.rearrange("(a k) n -> k a n", a=2))
            g1_sb = wp.tile([P, 2, 1], f32, name="g1_sb")
            nc.sync.dma_start(
                out=g1_sb[:], in_=ln1_g.rearrange("(a k) n -> k a n", a=2))
            b1_sb = wp.tile([P, 2, 1], f32, name="b1_sb")
            nc.sync.dma_start(
                out=b1_sb[:], in_=ln1_b.rearrange("(a k) n -> k a n", a=2))

            # ---------- LN1 (x in [t, dm] tiles) + transpose ----------
            xnT = xp.tile([P, 2, L], f32, name="xnT", tag="T2")  # [dm-tile, t]
            for i in range(8):  # t-tiles
                xt = wk.tile([P, D_MODEL], f32, name="xt", tag="xt")
                nc.sync.dma_start(out=xt[:], in_=x_in[i * P:(i + 1) * P, :])
                ssum = wk.tile([P, 1], f32, name="ssum", tag="ssum")
                nc.vector.tensor_reduce(ssum[:], xt[:],
                                        mybir.AxisListType.X, OP.add)
                sq = wk.tile([P, D_MODEL], f32, name="sq", tag="sq")
                sqsum = wk.tile([P, 1], f32, name="sqsum", tag="sqsum")
                nc.scalar.activation(sq[:], xt[:], AF.Square,
                                     accum_out=sqsum[:])
                mu = wk.tile([P, 1], f32, name="mu", tag="mu")
                nc.scalar.mul(mu[:], ssum[:], 1.0 / D_MODEL)
                mu2 = wk.tile([P, 1], f32, name="mu2", tag="mu2")
                nc.scalar.activation(mu2[:], mu[:], AF.Square)
                var = wk.tile([P, 1], f32, name="var", tag="var")
                nc.vector.scalar_tensor_tensor(
                    var[:], sqsum[:], 1.0 / D_MODEL, mu2[:], OP.mult,
                    OP.subtract)
                sd = wk.tile([P, 1], f32, name="sd", tag="sd")
                nc.scalar.activation(sd[:], var[:], AF.Sqrt, bias=eps_c[:])
                rs = wk.tile([P, 1], f32, name="rs", tag="rs")
                nc.vector.reciprocal(rs[:], sd[:])
                xm = wk.tile([P, D_MODEL], f32, name="xm", tag="xm")
                nc.vector.tensor_scalar(xm[:], xt[:], mu[:], None, OP.subtract)
                xs = wk.tile([P, D_MODEL], f32, name="xs", tag="xs")
                nc.vector.tensor_scalar(xs[:], xm[:], rs[:], None, OP.mult)
                for j in range(2):  # dm-tiles
                    tp = psA.tile([P, P], f32, name="tp", tag="br")
                    nc.tensor.transpose(tp[:], xs[:, j * P:(j + 1) * P],
                                        idt[:])
                    nc.scalar.activation(
                        xnT[:, j, i * P:(i + 1) * P], tp[:], AF.Identity,
                        bias=b1_sb[:, j, :], scale=g1_sb[:, j, :])

            # ---------- in-proj: xzT[p-tile, t] for 6 p-tiles ----------
            # p-tiles 0..3 = xi (d_inner), 4..5 = z_half
            xi = xp.tile([P, 4, L + 3], f32, name="xi", tag="T1")
            for j in range(4):
                nc.gpsimd.memset(xi[:, j, 0:3], 0.0)
            zs = xp.tile([P, 2, L], f32, name="zs")  # silu(z)
            for pt in range(6):
                for tcki in range(2):
                    ps = psA.tile([P, 512], f32, name="ps_inproj", tag="br")
                    for k in range(2):
                        nc.tensor.matmul(
                            ps[:], inw_sb[:, k, pt * P:(pt + 1) * P],
                            xnT[:, k, tcki * 512:(tcki + 1) * 512],
                            start=(k == 0), stop=(k == 1))
                    if pt < 4:
                        nc.vector.tensor_copy(
                            xi[:, pt, 3 + tcki * 512:3 + (tcki + 1) * 512],
                            ps[:])
                    else:
                        nc.scalar.activation(
                            zs[:, pt - 4, tcki * 512:(tcki + 1) * 512], ps[:],
                            AF.Silu)

            # ---------- conv + silu -> xc ----------
            xc = xp.tile([P, 4, L], f32, name="xc")
            for j in range(4):
                cv = wk.tile([P, L], f32, name="cv", tag="cv")
                nc.vector.tensor_scalar(cv[:], xi[:, j, 0:L],
                                        cw_sb[:, j, 0:1], None, OP.mult)
                for k in range(1, 4):
                    nc.vector.scalar_tensor_tensor(
                        cv[:], xi[:, j, k:L + k], cw_sb[:, j, k:k + 1], cv[:],
                        OP.mult, OP.add)
                nc.scalar.activation(xc[:, j, :], cv[:], AF.Silu,
                                     bias=cb_sb[:, j, :])

            # ---------- xproj -> dblT (dt16 B64 .. | C64 ..) ----------
            BT = xp.tile([D_STATE, L], f32, name="BT")
            CT = xp.tile([D_STATE, L], f32, name="CT")
            dtr = xp.tile([DT_RANK, L], f32, name="dtr")
            for pt in range(2):
                for tcki in range(2):
                    ps = psA.tile([P, 512], f32, name="ps_xproj", tag="br")
                    for k in range(4):
                        nc.tensor.matmul(
                            ps[:], xpj_sb[:, k, pt * P:(pt + 1) * P],
                            xc[:, k, tcki * 512:(tcki + 1) * 512],
                            start=(k == 0), stop=(k == 3))
                    sl = slice(tcki * 512, (tcki + 1) * 512)
                    if pt == 0:
                        nc.vector.tensor_copy(dtr[:, sl], ps[0:DT_RANK, :])
                        nc.vector.tensor_copy(BT[:, sl], ps[64:128, :])
                    else:
                        nc.vector.tensor_copy(CT[:, sl], ps[0:D_STATE, :])

            # ---------- dt = softplus(dtr @ dt_wT + dt_b); dtx ----------
            dt = xp.tile([P, 2, L], f32, name="dt")
            dtx = xp.tile([P, 2, L], f32, name="dtx")
            for j in range(2):
                for tcki in range(2):
                    ps = psA.tile([P, 512], f32, name="ps_dt", tag="br")
                    nc.tensor.matmul(
                        ps[:], dtw_sb[:, j * P:(j + 1) * P],
                        dtr[:, tcki * 512:(tcki + 1) * 512],
                        start=True, stop=True)
                    sl = slice(tcki * 512, (tcki + 1) * 512)
                    spt = wk.tile([P, 512], f32, name="spt", tag="spt")
                    nc.vector.tensor_scalar(spt[:], ps[:], dtb_sb[:, j, :],
                                            20.0, OP.add, OP.min)
                    spe = wk.tile([P, 512], f32, name="spe", tag="spe")
                    nc.scalar.activation(spe[:], spt[:], AF.Exp)
                    nc.scalar.activation(dt[:, j, sl], spe[:], AF.Ln,
                                         bias=1.0)
                # xc tile for this half: index half*2 + j -> host packs so
                # that xc tiles 0..3 are full d_inner; our half rows are
                # selected on host by reordering? No: we need xc[half].
                # Host passes xcsel index via weight layout instead: see
                # host prep - conv/xproj use full xc; for dtx/gate we use
                # xc tiles (HJ := half*2 + j) -- half is baked on host by
                # reordering in_wT xi columns? Simpler: host passes
                # hsel = half*2 selecting tiles; we hardcode via two
                # program variants? Instead: host reorders nothing;
                # we read xc tile (half*2+j) -- but half differs per core!
                # Resolution: host permutes in_w xi rows so that the
                # core's OWN half occupies tiles 0..1 (and the other half
                # tiles 2..3); conv_w/conv_b/xproj columns permuted to
                # match. Then our half is always tiles 0..1.
                nc.vector.tensor_tensor(dtx[:, j, :], dt[:, j, :],
                                        xc[:, j, :], OP.mult)

            # ---------- SSM scan core (n-outer, PE selector broadcast) ----------
            # GPS share of the g-mult: n in [0, GSPLIT)
            GSPLIT = 26
            yg = xp.tile([P, 2, L], f32, name="yg", tag="T1")
            yps = [psA.tile([P, 512], f32, name=f"yps_{j}_{t}",
                            tag=f"yps_{j}_{t}", bufs=1)
                   for j in range(2) for t in range(2)]
            for n in range(D_STATE):
                # Brep/Crep built on PE via selector matmul, staged to SBUF
                # by ACT so GPSIMD (no PSUM access) can read them.
                brs = wk.tile([P, L], f32, name="brs", tag="brs")
                for tcki in range(2):
                    br = psA.tile([P, 512], f32, name="br", tag="br")
                    nc.tensor.matmul(
                        br[:], id64_sb[:, n:n + 1].to_broadcast((D_STATE, P)),
                        BT[:, tcki * 512:(tcki + 1) * 512],
                        start=True, stop=True)
                    nc.scalar.activation(brs[:, tcki * 512:(tcki + 1) * 512],
                                         br[:], AF.Copy)
                crep = [None, None]
                crs = None
                if n < GSPLIT:
                    crs = wk.tile([P, L], f32, name="crs", tag="crs")
                for tcki in range(2):
                    cr = psA.tile([P, 512], f32, name="cr", tag="cr")
                    nc.tensor.matmul(
                        cr[:], id64_sb[:, n:n + 1].to_broadcast((D_STATE, P)),
                        CT[:, tcki * 512:(tcki + 1) * 512],
                        start=True, stop=True)
                    if n < GSPLIT:
                        nc.scalar.activation(
                            crs[:, tcki * 512:(tcki + 1) * 512], cr[:],
                            AF.Copy)
                    crep[tcki] = cr
                for j in range(2):
                    dA = wk.tile([P, L], f32, name="dA", tag="dA")
                    nc.scalar.activation(dA[:], dt[:, j, :], AF.Exp,
                                         scale=A_sb[:, j, n:n + 1])
                    dBx = wk.tile([P, L], f32, name="dBx", tag="dBx")
                    nc.gpsimd.tensor_tensor(dBx[:], dtx[:, j, :], brs[:],
                                            OP.mult)
                    h = wk.tile([P, L], f32, name="h", tag="h")
                    nc.vector.tensor_tensor_scan(
                        h[:], dA[:], dBx[:], 0.0, OP.mult, OP.add)
                    g = wk.tile([P, L], f32, name="g", tag="g")
                    if n < GSPLIT:
                        nc.gpsimd.tensor_tensor(g[:], h[:], crs[:], OP.mult)
                    else:
                        for tcki in range(2):
                            nc.vector.tensor_tensor(
                                g[:, tcki * 512:(tcki + 1) * 512],
                                h[:, tcki * 512:(tcki + 1) * 512],
                                crep[tcki][:], OP.mult)
                    for tcki in range(2):
                        nc.tensor.matmul(
                            yps[j * 2 + tcki][:], idt[:],
                            g[:, tcki * 512:(tcki + 1) * 512],
                            start=(n == 0), stop=(n == D_STATE - 1))
            # gate: y = (y + xc*Dp) * silu(z)
            for j in range(2):
                for tcki in range(2):
                    sl = slice(tcki * 512, (tcki + 1) * 512)
                    yt = wk.tile([P, 512], f32, name="yt", tag="yt")
                    nc.vector.scalar_tensor_tensor(
                        yt[:], xc[:, j, sl], Dp_sb[:, j, :],
                        yps[j * 2 + tcki][:], OP.mult, OP.add)
                    nc.vector.tensor_tensor(yg[:, j, sl], yt[:],
                                            zs[:, j, sl], OP.mult)

            # ---------- out-proj ----------
            pout = xp.tile([P, 2, L], f32, name="pout", tag="T2")
            for pt in range(2):
                for tcki in range(2):
                    ps = psA.tile([P, 512], f32, name="ps_out", tag="br")
                    for k in range(2):
                        nc.tensor.matmul(
                            ps[:], ow_sb[:, k, pt * P:(pt + 1) * P],
                            yg[:, k, tcki * 512:(tcki + 1) * 512],
                            start=(k == 0), stop=(k == 1))
                    nc.vector.tensor_copy(
                        pout[:, pt, tcki * 512:(tcki + 1) * 512], ps[:])
            nc.sync.dma_start(
                out=part.rearrange("(a k) n -> k a n", a=2), in_=pout[:])

    nc.compile()
    return nc


def _get_runner():
    if "run" not in _cache:
        import jax
        import numpy as _np
        from jax.sharding import Mesh, PartitionSpec
        from jax.experimental.shard_map import shard_map
        import concourse.mybir as mybir
        from concourse.bass2jax import (_bass_exec_p, install_neuronx_cc_hook,
                                        partition_id_tensor)

        nc = _build()
        install_neuronx_cc_hook()
        partition_name = (nc.partition_id_tensor.name
                          if nc.partition_id_tensor else None)
        in_names, out_names, out_avals = [], [], []
        for alloc in nc.m.functions[0].allocations:
            if not isinstance(alloc, mybir.MemoryLocationSet):
                continue
            name = alloc.memorylocations[0].name
            if alloc.kind == "ExternalInput":
                if name != partition_name:
                    in_names.append(name)
            elif alloc.kind == "ExternalOutput":
                out_names.append(name)
                out_avals.append(jax.core.ShapedArray(
                    tuple(alloc.tensor_shape), mybir.dt.np(alloc.dtype)))
        n_params = len(in_names)
        n_outs = len(out_avals)
        all_in = list(in_names) + list(out_names)
        if partition_name is not None:
            all_in.append(partition_name)

        def _body(*args):
            operands = list(args)
            if partition_name is not None:
                operands.append(partition_id_tensor())
            return tuple(_bass_exec_p.bind(
                *operands, out_avals=tuple(out_avals),
                in_names=tuple(all_in), out_names=tuple(out_names),
                lowering_input_output_aliases=(),
                sim_require_finite=True, sim_require_nnan=True, nc=nc))

        devices = jax.devices()[:8]
        mesh = Mesh(_np.asarray(devices), ("core",))
        sharded = jax.jit(
            shard_map(_body, mesh=mesh,
                      in_specs=(PartitionSpec("core"),) * (n_params + n_outs),
                      out_specs=(PartitionSpec("core"),) * n_outs,
                      check_rep=False),
            keep_unused=True)

        def run(in_maps):
            per_core = [[_np.asarray(m[name]) for name in in_names]
                        for m in in_maps]
            concat_in = [
                _np.concatenate([per_core[c][i] for c in range(8)], axis=0)
                for i in range(n_params)]
            concat_zeros = [_np.zeros((8 * a.shape[0], *a.shape[1:]), a.dtype)
                            for a in out_avals]
            out = sharded(*concat_in, *concat_zeros)
            jax.block_until_ready(out)
            return [
                {name: _np.asarray(out[i]).reshape(8, *out_avals[i].shape)[c]
                 for i, name in enumerate(out_names)}
                for c in range(8)]

        _cache["run"] = run
    return _cache["run"]


def _prep_core_inputs(inputs, b, direction, half):
    """Host-side shard prep for one core. direction: 0 fwd, 1 bwd."""
    pre = "f_" if direction == 0 else "b_"
    g = lambda k: np.asarray(inputs[pre + k], np.float32)

    hs = slice(half * HALF, (half + 1) * HALF)
    oh = slice((1 - half) * HALF, (2 - half) * HALF)
    # permute d_inner so the core's own half occupies rows 0:256
    perm = np.r_[half * HALF:(half + 1) * HALF,
                 (1 - half) * HALF:(2 - half) * HALF]

    x = np.asarray(inputs["x"], np.float32)[b]
    if direction == 1:
        x = x[::-1]

    in_w = g("in_w")            # [1024, 256]
    xi_w = in_w[:D_INNER][perm]            # [512, 256] permuted
    z_w = in_w[D_INNER:][hs]               # [256, 256] own half
    in_wT = np.concatenate([xi_w, z_w], axis=0).T.copy()  # [256, 768]

    xproj = g("xproj_w")        # [144, 512]
    xproj_p = xproj[:, perm]               # permute input cols
    blk = np.zeros((256, D_INNER), np.float32)
    blk[0:16] = xproj_p[0:16]
    blk[64:128] = xproj_p[16:80]
    blk[128:192] = xproj_p[80:144]
    xprojT = blk.T.copy()                  # [512, 256]

    conv = g("conv_w").reshape(D_INNER, D_CONV)[perm]
    convb = g("conv_b")[perm].reshape(D_INNER, 1)
    dt_w = g("dt_w")            # [512, 16]
    dt_wT = dt_w[hs].T.copy()              # [16, 256]
    dtb = g("dt_b")[hs].reshape(HALF, 1)
    A = -np.exp(g("A_log"))[hs]            # [256, 64]
    Dp = g("Dp")[hs].reshape(HALF, 1)
    out_w = g("out_w")          # [256, 512]
    out_wT = out_w[:, hs].T.copy()         # [256, 256]

    return {
        "x_in": np.ascontiguousarray(x),
        "in_wT": np.ascontiguousarray(in_wT),
        "xprojT": np.ascontiguousarray(xprojT),
        "dt_wT": np.ascontiguousarray(dt_wT),
        "conv_w": np.ascontiguousarray(conv),
        "conv_b": convb,
        "dt_b": dtb,
        "A_in": np.ascontiguousarray(A),
        "Dp_in": Dp,
        "out_wT": np.ascontiguousarray(out_wT),
        "ln1_g": np.asarray(inputs["ln1_g"], np.float32).reshape(-1, 1),
        "ln1_b": np.asarray(inputs["ln1_b"], np.float32).reshape(-1, 1),
        "ident": np.eye(P, dtype=np.float32),
        "ones1": np.ones((1, P), np.float32),
        "id64_in": np.eye(D_STATE, dtype=np.float32),
    }


def kernel(**inputs):
    run = _get_runner()
    in_maps = []
    for c in range(8):
        b, direction, half = c >> 2, (c >> 1) & 1, c & 1
        in_maps.append(_prep_core_inputs(inputs, b, direction, half))
    outs = run(in_maps)

    # host: gather partials -> x_ssm -> LN2 -> w2 -> gelu
    x_ssm = np.zeros((BATCH, L, D_MODEL), np.float32)
    for c in range(8):
        b, direction = c >> 2, (c >> 1) & 1
        p = outs[c]["part"].T  # [t, dm]
        if direction == 1:
            p = p[::-1]
        x_ssm[b] += p

    mu = x_ssm.mean(-1, keepdims=True)
    var = x_ssm.var(-1, keepdims=True)
    ln2_g = np.asarray(inputs["ln2_g"], np.float32)
    ln2_b = np.asarray(inputs["ln2_b"], np.float32)
    x2 = (x_ssm - mu) / np.sqrt(var + LN_EPS) * ln2_g + ln2_b
    w2 = np.asarray(inputs["w2"], np.float32)
    b2 = np.asarray(inputs["b2"], np.float32)
    z = x2 @ w2.T + b2
    from scipy.special import erf
    out = 0.5 * z * (1.0 + erf(z / np.sqrt(2.0).astype(np.float32)))
    return out.astype(np.float32)


# revision 19
# speedup vs baseline: 1083.5759x; 1083.5759x over previous
"""BiMamba encoder block on 8 trn2 NeuronCores.

Sharding: core = (batch b in {0,1}) x (direction in {fwd,bwd}) x
(d_inner half in {0,1}).  Each core runs the same Bass program on its own
shard: LN1 -> in-proj -> depthwise causal conv -> silu -> x-proj ->
dt/softplus -> selective scan (native DVE tensor_tensor_scan per
(d-tile, n)) -> gated output projection partial.  Host sums the four
partials per batch (bwd cores process a host-flipped sequence) and
applies LN2 + w2 + exact GELU.
"""
import numpy as np

D_MODEL = 256
D_STATE = 64
D_CONV = 4
D_INNER = 512
DT_RANK = 16
BATCH = 2
SEQ = 1024
LN_EPS = 1e-5

HALF = D_INNER // 2  # 256 channels per core
P = 128
L = SEQ

_cache = {}



def _build(iters=1, variant="full"):
    import concourse.bacc as bacc
    import concourse.mybir as mybir
    from concourse.tile import TileContext

    f32 = mybir.dt.float32
    AF = mybir.ActivationFunctionType
    OP = mybir.AluOpType

    nc = bacc.Bacc("TRN2", target_bir_lowering=False, debug=False,
                   num_devices=8)

    # ---- per-core inputs (host-prepped) ----
    x_in = nc.declare_dram_parameter("x_in", [L, D_MODEL], f32, isOutput=False)
    in_wT = nc.declare_dram_parameter("in_wT", [D_MODEL, 768], f32,
                                      isOutput=False)  # cols: xi(512), z_half(256)
    xprojT = nc.declare_dram_parameter("xprojT", [D_INNER, 256], f32,
                                       isOutput=False)  # [dt16 B64 0*48 | C64 0*64]
    dt_wT = nc.declare_dram_parameter("dt_wT", [DT_RANK, HALF], f32,
                                      isOutput=False)
    conv_w = nc.declare_dram_parameter("conv_w", [D_INNER, D_CONV], f32,
                                       isOutput=False)
    conv_b = nc.declare_dram_parameter("conv_b", [D_INNER, 1], f32,
                                       isOutput=False)
    dt_b = nc.declare_dram_parameter("dt_b", [HALF, 1], f32, isOutput=False)
    A_in = nc.declare_dram_parameter("A_in", [HALF, D_STATE], f32,
                                     isOutput=False)
    Dp_in = nc.declare_dram_parameter("Dp_in", [HALF, 1], f32, isOutput=False)
    out_wT = nc.declare_dram_parameter("out_wT", [HALF, D_MODEL], f32,
                                       isOutput=False)
    ln1_g = nc.declare_dram_parameter("ln1_g", [D_MODEL, 1], f32,
                                      isOutput=False)
    ln1_b = nc.declare_dram_parameter("ln1_b", [D_MODEL, 1], f32,
                                      isOutput=False)
    ident = nc.declare_dram_parameter("ident", [P, P], f32, isOutput=False)
    ones1 = nc.declare_dram_parameter("ones1", [1, P], f32, isOutput=False)
    id64_in = nc.declare_dram_parameter("id64_in", [D_STATE, D_STATE], f32,
                                        isOutput=False)

    part = nc.declare_dram_parameter("part", [D_MODEL, L], f32, isOutput=True)

    from contextlib import nullcontext
    with TileContext(nc) as tc:
        with tc.tile_pool(name="wpool", bufs=1) as wp, \
             tc.tile_pool(name="xpool", bufs=1) as xp, \
             tc.tile_pool(name="work", bufs=2) as wk, \
             tc.tile_pool(name="psA", bufs=2, space="PSUM") as psA, \
             (tc.For_i(0, iters, 1) if iters > 1 else nullcontext()):

            # ---------- load weights ----------
            eps_c = wp.tile([P, 1], f32, name="eps_c")
            nc.gpsimd.memset(eps_c[:], LN_EPS)
            idt = wp.tile([P, P], f32, name="idt")
            nc.sync.dma_start(out=idt[:], in_=ident[:])
            ones_sb = wp.tile([1, P], f32, name="ones_sb")
            nc.sync.dma_start(out=ones_sb[:], in_=ones1[:])
            id64_sb = wp.tile([D_STATE, D_STATE], f32, name="id64_sb")
            nc.sync.dma_start(out=id64_sb[:], in_=id64_in[:])
            inw_sb = wp.tile([P, 2, 768], f32, name="inw_sb")  # [k-chunk][...]
            nc.sync.dma_start(
                out=inw_sb[:], in_=in_wT.rearrange("(a k) n -> k a n", a=2))
            xpj_sb = wp.tile([P, 4, 256], f32, name="xpj_sb")
            nc.sync.dma_start(
                out=xpj_sb[:], in_=xprojT.rearrange("(a k) n -> k a n", a=4))
            dtw_sb = wp.tile([DT_RANK, HALF], f32, name="dtw_sb")
            nc.sync.dma_start(out=dtw_sb[:], in_=dt_wT[:])
            cw_sb = wp.tile([P, 4, D_CONV], f32, name="cw_sb")
            nc.sync.dma_start(
                out=cw_sb[:], in_=conv_w.rearrange("(a k) n -> k a n", a=4))
            cb_sb = wp.tile([P, 4, 1], f32, name="cb_sb")
            nc.sync.dma_start(
                out=cb_sb[:], in_=conv_b.rearrange("(a k) n -> k a n", a=4))
            dtb_sb = wp.tile([P, 2, 1], f32, name="dtb_sb")
            nc.sync.dma_start(
                out=dtb_sb[:], in_=dt_b.rearrange("(a k) n -> k a n", a=2))
            A_sb = wp.tile([P, 2, D_STATE], f32, name="A_sb")
            nc.sync.dma_start(
                out=A_sb[:], in_=A_in.rearrange("(a k) n -> k a n", a=2))
            Dp_sb = wp.tile([P, 2, 1], f32, name="Dp_sb")
            nc.sync.dma_start(
                out=Dp_sb[:], in_=Dp_in.rearrange("(a k) n -> k a n", a=2))
            ow_sb = wp.tile([P, 2, D_MODEL], f32, name="ow_sb")
            nc.sync.dma_start(
                out=ow_sb[:], in_=out_wT.rearrange("(a k) n -> k a n", a=2))
            g1_sb = wp.tile([P, 2, 1], f32, name="g1_sb")
            nc.sync.dma_start(
                out=g1_sb[:], in_=ln1_g.rearrange("(a k) n -> k a n", a=2))
            b1_sb = wp.tile([P, 2, 1], f32, name="b1_sb")
            nc.sync.dma_start(
                out=b1_sb[:], in_=ln1_b.rearrange("(a k) n -> k a n", a=2))

            # ---------- LN1 (x in [t, dm] tiles) + transpose ----------
            xnT = xp.tile([P, 2, L], f32, name="xnT", tag="T2")  # [dm-tile, t]
            for i in range(8):  # t-tiles
                xt = wk.tile([P, D_MODEL], f32, name="xt", tag="xt")
                nc.sync.dma_start(out=xt[:], in_=x_in[i * P:(i + 1) * P, :])
                ssum = wk.tile([P, 1], f32, name="ssum", tag="ssum")
                nc.vector.tensor_reduce(ssum[:], xt[:],
                                        mybir.AxisListType.X, OP.add)
                sq = wk.tile([P, D_MODEL], f32, name="sq", tag="sq")
                sqsum = wk.tile([P, 1], f32, name="sqsum", tag="sqsum")
                nc.scalar.activation(sq[:], xt[:], AF.Square,
                                     accum_out=sqsum[:])
                mu = wk.tile([P, 1], f32, name="mu", tag="mu")
                nc.scalar.mul(mu[:], ssum[:], 1.0 / D_MODEL)
                mu2 = wk.tile([P, 1], f32, name="mu2", tag="mu2")
                nc.scalar.activation(mu2[:], mu[:], AF.Square)
                var = wk.tile([P, 1], f32, name="var", tag="var")
                nc.vector.scalar_tensor_tensor(
                    var[:], sqsum[:], 1.0 / D_MODEL, mu2[:], OP.mult,
                    OP.subtract)
                sd = wk.tile([P, 1], f32, name="sd", tag="sd")
                nc.scalar.activation(sd[:], var[:], AF.Sqrt, bias=eps_c[:])
                rs = wk.tile([P, 1], f32, name="rs", tag="rs")
                nc.vector.reciprocal(rs[:], sd[:])
                xm = wk.tile([P, D_MODEL], f32, name="xm", tag="xm")
                nc.vector.tensor_scalar(xm[:], xt[:], mu[:], None, OP.subtract)
                xs = wk.tile([P, D_MODEL], f32, name="xs", tag="xs")
                nc.vector.tensor_scalar(xs[:], xm[:], rs[:], None, OP.mult)
                for j in range(2):  # dm-tiles
                    tp = psA.tile([P, P], f32, name="tp", tag="br")
                    nc.tensor.transpose(tp[:], xs[:, j * P:(j + 1) * P],
                                        idt[:])
                    nc.scalar.activation(
                        xnT[:, j, i * P:(i + 1) * P], tp[:], AF.Identity,
                        bias=b1_sb[:, j, :], scale=g1_sb[:, j, :])

            # ---------- in-proj: xzT[p-tile, t] for 6 p-tiles ----------
            # p-tiles 0..3 = xi (d_inner), 4..5 = z_half
            xi = xp.tile([P, 4, L + 3], f32, name="xi", tag="T1")
            for j in range(4):
                nc.gpsimd.memset(xi[:, j, 0:3], 0.0)
            zs = xp.tile([P, 2, L], f32, name="zs")  # silu(z)
            for pt in range(6):
                for tcki in range(2):
                    ps = psA.tile([P, 512], f32, name="ps_inproj", tag="br")
                    for k in range(2):
                        nc.tensor.matmul(
                            ps[:], inw_sb[:, k, pt * P:(pt + 1) * P],
                            xnT[:, k, tcki * 512:(tcki + 1) * 512],
                            start=(k == 0), stop=(k == 1))
                    if pt < 4:
                        nc.vector.tensor_copy(
                            xi[:, pt, 3 + tcki * 512:3 + (tcki + 1) * 512],
                            ps[:])
                    else:
                        nc.scalar.activation(
                            zs[:, pt - 4, tcki * 512:(tcki + 1) * 512], ps[:],
                            AF.Silu)

            # ---------- conv + silu -> xc ----------
            xc = xp.tile([P, 4, L], f32, name="xc")
            for j in range(4):
                cv = wk.tile([P, L], f32, name="cv", tag="cv")
                nc.vector.tensor_scalar(cv[:], xi[:, j, 0:L],
                                        cw_sb[:, j, 0:1], None, OP.mult)
                for k in range(1, 4):
                    nc.vector.scalar_tensor_tensor(
                        cv[:], xi[:, j, k:L + k], cw_sb[:, j, k:k + 1], cv[:],
                        OP.mult, OP.add)
                nc.scalar.activation(xc[:, j, :], cv[:], AF.Silu,
                                     bias=cb_sb[:, j, :])

            # ---------- xproj -> dblT (dt16 B64 .. | C64 ..) ----------
            BT = xp.tile([D_STATE, L], f32, name="BT")
            CT = xp.tile([D_STATE, L], f32, name="CT")
            dtr = xp.tile([DT_RANK, L], f32, name="dtr")
            for pt in range(2):
                for tcki in range(2):
                    ps = psA.tile([P, 512], f32, name="ps_xproj", tag="br")
                    for k in range(4):
                        nc.tensor.matmul(
                            ps[:], xpj_sb[:, k, pt * P:(pt + 1) * P],
                            xc[:, k, tcki * 512:(tcki + 1) * 512],
                            start=(k == 0), stop=(k == 3))
                    sl = slice(tcki * 512, (tcki + 1) * 512)
                    if pt == 0:
                        nc.vector.tensor_copy(dtr[:, sl], ps[0:DT_RANK, :])
                        nc.vector.tensor_copy(BT[:, sl], ps[64:128, :])
                    else:
                        nc.vector.tensor_copy(CT[:, sl], ps[0:D_STATE, :])

            # ---------- dt = softplus(dtr @ dt_wT + dt_b); dtx ----------
            dt = xp.tile([P, 2, L], f32, name="dt")
            dtx = xp.tile([P, 2, L], f32, name="dtx")
            for j in range(2):
                for tcki in range(2):
                    ps = psA.tile([P, 512], f32, name="ps_dt", tag="br")
                    nc.tensor.matmul(
                        ps[:], dtw_sb[:, j * P:(j + 1) * P],
                        dtr[:, tcki * 512:(tcki + 1) * 512],
                        start=True, stop=True)
                    sl = slice(tcki * 512, (tcki + 1) * 512)
                    spt = wk.tile([P, 512], f32, name="spt", tag="spt")
                    nc.vector.tensor_scalar(spt[:], ps[:], dtb_sb[:, j, :],
                                            20.0, OP.add, OP.min)
                    spe = wk.tile([P, 512], f32, name="spe", tag="spe")
                    nc.scalar.activation(spe[:], spt[:], AF.Exp)
                    nc.scalar.activation(dt[:, j, sl], spe[:], AF.Ln,
                                         bias=1.0)
                # xc tile for this half: index half*2 + j -> host packs so
                # that xc tiles 0..3 are full d_inner; our half rows are
                # selected on host by reordering? No: we need xc[half].
                # Host passes xcsel index via weight layout instead: see
                # host prep - conv/xproj use full xc; for dtx/gate we use
                # xc tiles (HJ := half*2 + j) -- half is baked on host by
                # reordering in_wT xi columns? Simpler: host passes
                # hsel = half*2 selecting tiles; we hardcode via two
                # program variants? Instead: host reorders nothing;
                # we read xc tile (half*2+j) -- but half differs per core!
                # Resolution: host permutes in_w xi rows so that the
                # core's OWN half occupies tiles 0..1 (and the other half
                # tiles 2..3); conv_w/conv_b/xproj columns permuted to
                # match. Then our half is always tiles 0..1.
                nc.vector.tensor_tensor(dtx[:, j, :], dt[:, j, :],
                                        xc[:, j, :], OP.mult)

            # ---------- SSM scan core (n-outer, shared PE selectors) ----------
            bf = mybir.dt.bfloat16
            RB = (variant == "bf16r")
            gdt = bf if RB else f32
            yg = xp.tile([P, 2, L], f32, name="yg", tag="T1")
            yps = [psA.tile([P, 512], f32, name=f"yps_{j}_{t}",
                            tag=f"yps_{j}_{t}", bufs=1)
                   for j in range(2) for t in range(2)]
            for n in range(D_STATE):
                brep = [None, None]
                crep = [None, None]
                crs = None
                if RB:
                    crs = wk.tile([P, L], bf, name="crs", tag="crs")
                for tcki in range(2):
                    br = psA.tile([P, 512], f32, name="br", tag="br")
                    nc.tensor.matmul(
                        br[:], id64_sb[:, n:n + 1].to_broadcast((D_STATE, P)),
                        BT[:, tcki * 512:(tcki + 1) * 512],
                        start=True, stop=True)
                    brep[tcki] = br
                    cr = psA.tile([P, 512], f32, name="cr", tag="cr")
                    nc.tensor.matmul(
                        cr[:], id64_sb[:, n:n + 1].to_broadcast((D_STATE, P)),
                        CT[:, tcki * 512:(tcki + 1) * 512],
                        start=True, stop=True)
                    if RB:
                        nc.scalar.activation(
                            crs[:, tcki * 512:(tcki + 1) * 512], cr[:],
                            AF.Copy)
                    crep[tcki] = cr
                for j in range(2):
                    dA = wk.tile([P, L], f32, name="dA", tag="dA")
                    nc.scalar.activation(dA[:], dt[:, j, :], AF.Exp,
                                         scale=A_sb[:, j, n:n + 1])
                    dBx = wk.tile([P, L], f32, name="dBx", tag="dBx")
                    for tcki in range(2):
                        nc.vector.tensor_tensor(
                            dBx[:, tcki * 512:(tcki + 1) * 512],
                            dtx[:, j, tcki * 512:(tcki + 1) * 512],
                            brep[tcki][:], OP.mult)
                    h = wk.tile([P, L], gdt, name="h", tag="h")
                    nc.vector.tensor_tensor_scan(
                        h[:], dA[:], dBx[:], 0.0, OP.mult, OP.add)
                    g = wk.tile([P, L], gdt, name="g", tag="g")
                    if RB:
                        nc.vector.tensor_tensor(g[:], h[:], crs[:], OP.mult)
                    else:
                        for tcki in range(2):
                            nc.vector.tensor_tensor(
                                g[:, tcki * 512:(tcki + 1) * 512],
                                h[:, tcki * 512:(tcki + 1) * 512],
                                crep[tcki][:], OP.mult)
                    for tcki in range(2):
                        nc.tensor.matmul(
                            yps[j * 2 + tcki][:], idt[:],
                            g[:, tcki * 512:(tcki + 1) * 512],
                            start=(n == 0), stop=(n == D_STATE - 1))
            # gate: y = (y + xc*Dp) * silu(z)
            for j in range(2):
                for tcki in range(2):
                    sl = slice(tcki * 512, (tcki + 1) * 512)
                    yt = wk.tile([P, 512], f32, name="yt", tag="yt")
                    nc.vector.scalar_tensor_tensor(
                        yt[:], xc[:, j, sl], Dp_sb[:, j, :],
                        yps[j * 2 + tcki][:], OP.mult, OP.add)
                    nc.vector.tensor_tensor(yg[:, j, sl], yt[:],
                                            zs[:, j, sl], OP.mult)

            # ---------- out-proj ----------
            pout = xp.tile([P, 2, L], f32, name="pout", tag="T2")
            for pt in range(2):
                for tcki in range(2):
                    ps = psA.tile([P, 512], f32, name="ps_out", tag="br")
                    for k in range(2):
                        nc.tensor.matmul(
                            ps[:], ow_sb[:, k, pt * P:(pt + 1) * P],
                            yg[:, k, tcki * 512:(tcki + 1) * 512],
                            start=(k == 0), stop=(k == 1))
                    nc.vector.tensor_copy(
                        pout[:, pt, tcki * 512:(tcki + 1) * 512], ps[:])
            nc.sync.dma_start(
                out=part.rearrange("(a k) n -> k a n", a=2), in_=pout[:])

    nc.compile()
    return nc


def _get_runner():
    if "run" not in _cache:
        import jax
        import numpy as _np
        from jax.sharding import Mesh, PartitionSpec
        from jax.experimental.shard_map import shard_map
        import concourse.mybir as mybir
        from concourse.bass2jax import (_bass_exec_p, install_neuronx_cc_hook,
                                        partition_id_tensor)

        nc = _build()
        install_neuronx_cc_hook()
        partition_name = (nc.partition_id_tensor.name
                          if nc.partition_id_tensor else None)
        in_names, out_names, out_avals = [], [], []
        for alloc in nc.m.functions[0].allocations:
            if not isinstance(alloc, mybir.MemoryLocationSet):
                continue
            name = alloc.memorylocations[0].name
            if alloc.kind == "ExternalInput":
                if name != partition_name:
                    in_names.append(name)
            elif alloc.kind == "ExternalOutput":
                out_names.append(name)
                out_avals.append(jax.core.ShapedArray(
                    tuple(alloc.tensor_shape), mybir.dt.np(alloc.dtype)))
        n_params = len(in_names)
        n_outs = len(out_avals)
        all_in = list(in_names) + list(out_names)
        if partition_name is not None:
            all_in.append(partition_name)

        def _body(*args):
            operands = list(args)
            if partition_name is not None:
                operands.append(partition_id_tensor())
            return tuple(_bass_exec_p.bind(
                *operands, out_avals=tuple(out_avals),
                in_names=tuple(all_in), out_names=tuple(out_names),
                lowering_input_output_aliases=(),
                sim_require_finite=True, sim_require_nnan=True, nc=nc))

        devices = jax.devices()[:8]
        mesh = Mesh(_np.asarray(devices), ("core",))
        sharded = jax.jit(
            shard_map(_body, mesh=mesh,
                      in_specs=(PartitionSpec("core"),) * (n_params + n_outs),
                      out_specs=(PartitionSpec("core"),) * n_outs,
                      check_rep=False),
            keep_unused=True)

        def run(in_maps):
            per_core = [[_np.asarray(m[name]) for name in in_names]
                        for m in in_maps]
            concat_in = [
                _np.concatenate([per_core[c][i] for c in range(8)], axis=0)
                for i in range(n_params)]
            concat_zeros = [_np.zeros((8 * a.shape[0], *a.shape[1:]), a.dtype)
                            for a in out_avals]
            out = sharded(*concat_in, *concat_zeros)
            jax.block_until_ready(out)
            return [
                {name: _np.asarray(out[i]).reshape(8, *out_avals[i].shape)[c]
                 for i, name in enumerate(out_names)}
                for c in range(8)]

        _cache["run"] = run
    return _cache["run"]


def _prep_core_inputs(inputs, b, direction, half):
    """Host-side shard prep for one core. direction: 0 fwd, 1 bwd."""
    pre = "f_" if direction == 0 else "b_"
    g = lambda k: np.asarray(inputs[pre + k], np.float32)

    hs = slice(half * HALF, (half + 1) * HALF)
    oh = slice((1 - half) * HALF, (2 - half) * HALF)
    # permute d_inner so the core's own half occupies rows 0:256
    perm = np.r_[half * HALF:(half + 1) * HALF,
                 (1 - half) * HALF:(2 - half) * HALF]

    x = np.asarray(inputs["x"], np.float32)[b]
    if direction == 1:
        x = x[::-1]

    in_w = g("in_w")            # [1024, 256]
    xi_w = in_w[:D_INNER][perm]            # [512, 256] permuted
    z_w = in_w[D_INNER:][hs]               # [256, 256] own half
    in_wT = np.concatenate([xi_w, z_w], axis=0).T.copy()  # [256, 768]

    xproj = g("xproj_w")        # [144, 512]
    xproj_p = xproj[:, perm]               # permute input cols
    blk = np.zeros((256, D_INNER), np.float32)
    blk[0:16] = xproj_p[0:16]
    blk[64:128] = xproj_p[16:80]
    blk[128:192] = xproj_p[80:144]
    xprojT = blk.T.copy()                  # [512, 256]

    conv = g("conv_w").reshape(D_INNER, D_CONV)[perm]
    convb = g("conv_b")[perm].reshape(D_INNER, 1)
    dt_w = g("dt_w")            # [512, 16]
    dt_wT = dt_w[hs].T.copy()              # [16, 256]
    dtb = g("dt_b")[hs].reshape(HALF, 1)
    A = -np.exp(g("A_log"))[hs]            # [256, 64]
    Dp = g("Dp")[hs].reshape(HALF, 1)
    out_w = g("out_w")          # [256, 512]
    out_wT = out_w[:, hs].T.copy()         # [256, 256]

    return {
        "x_in": np.ascontiguousarray(x),
        "in_wT": np.ascontiguousarray(in_wT),
        "xprojT": np.ascontiguousarray(xprojT),
        "dt_wT": np.ascontiguousarray(dt_wT),
        "conv_w": np.ascontiguousarray(conv),
        "conv_b": convb,
        "dt_b": dtb,
        "A_in": np.ascontiguousarray(A),
        "Dp_in": Dp,
        "out_wT": np.ascontiguousarray(out_wT),
        "ln1_g": np.asarray(inputs["ln1_g"], np.float32).reshape(-1, 1),
        "ln1_b": np.asarray(inputs["ln1_b"], np.float32).reshape(-1, 1),
        "ident": np.eye(P, dtype=np.float32),
        "ones1": np.ones((1, P), np.float32),
        "id64_in": np.eye(D_STATE, dtype=np.float32),
    }


def kernel(**inputs):
    run = _get_runner()
    in_maps = []
    for c in range(8):
        b, direction, half = c >> 2, (c >> 1) & 1, c & 1
        in_maps.append(_prep_core_inputs(inputs, b, direction, half))
    outs = run(in_maps)

    # host: gather partials -> x_ssm -> LN2 -> w2 -> gelu
    x_ssm = np.zeros((BATCH, L, D_MODEL), np.float32)
    for c in range(8):
        b, direction = c >> 2, (c >> 1) & 1
        p = outs[c]["part"].T  # [t, dm]
        if direction == 1:
            p = p[::-1]
        x_ssm[b] += p

    mu = x_ssm.mean(-1, keepdims=True)
    var = x_ssm.var(-1, keepdims=True)
    ln2_g = np.asarray(inputs["ln2_g"], np.float32)
    ln2_b = np.asarray(inputs["ln2_b"], np.float32)
    x2 = (x_ssm - mu) / np.sqrt(var + LN_EPS) * ln2_g + ln2_b
    w2 = np.asarray(inputs["w2"], np.float32)
    b2 = np.asarray(inputs["b2"], np.float32)
    z = x2 @ w2.T + b2
    from scipy.special import erf
    out = 0.5 * z * (1.0 + erf(z / np.sqrt(2.0).astype(np.float32)))
    return out.astype(np.float32)
